# revision 4
# baseline (speedup 1.0000x reference)
"""Trainium2 Bass kernel for MultiLinearAttention (causal linear attention).

Reference computation (per head h, feature map phi(u) = elu(u)+1):
    q = phi(x_h @ Wq_h), k = phi(x_h @ Wk_h), v = x_h @ Wv_h
    y_t = (q_t . sum_{s<=t} k_s v_s^T) / (q_t . sum_{s<=t} k_s + eps)
    out = concat_h(y_h) @ Wp

Sharding: 16 heads / 8 cores = 2 heads per core, all 4 batches per core.
Wp is folded per-head into the v projection (W'_h = Wv_h @ Wp_h), so each
core produces per-head partial numerators [B, S, 2, 64] plus per-head
denominators [B, S, 2]; the host unshard computes
    y = sum_cores sum_h num_h / (den_h + eps).

Device algorithm: chunked causal linear attention, chunk C=128, with all
4 batches fused per chunk into wide ops:
    u = 1 + [q|k] projections (PSUM preset via K=1 ones matmul)
    phi = max(u, min(exp(u-1), 1)) == elu(.)+1
    A^T = K Q^T per (b,h) (8 blocks, layout h-major: [4xh0 | 4xh1])
    am = A ⊙ causal-mask (one DVE op over all 8 blocks)
    num = am^T V + Q S_prev  (per-batch 128-col blocks, fp32 PSUM)
    den = am^T 1 + Q z_prev  (separate [128,8+4] PSUM tile)
    S += Kt^T V; z_chunk = Kt^T 1 accumulated into SBUF z-sum.

PSUM bank budget (8 banks): state[512]f32 | u[1024]f32 x2 | A[1024]f32 x2 |
num/vk time-shared [512]f32 | knp[512]bf16 | den+z [12]f32.
"""

import os
import sys

import numpy as np

for _p in ("/root/.axon_site/_ro/trn_rl_repo", "/opt/trn_rl_repo", "/opt/pypackages"):
    if os.path.isdir(_p) and _p not in sys.path:
        sys.path.append(_p)

import ml_dtypes

B, S, D = 4, 4096, 1024
H, HD, O = 16, 64, 64
C = 128                  # chunk length
NCORE = 8
HPC = H // NCORE         # heads per core
NCHUNK = S // C
EPS = 1e-6

_CACHE = {}


def _build_program(nchunk=NCHUNK):
    import concourse.mybir as mybir
    from concourse import bacc
    from concourse.tile import TileContext

    fp32 = mybir.dt.float32
    cdt = mybir.dt.bfloat16
    Alu = mybir.AluOpType
    Act = mybir.ActivationFunctionType

    nc = bacc.Bacc()
    # x staged feature-major, chunk-interleaved: col = 512*chunk + 128*b + s
    xq_h = nc.declare_dram_parameter("xq", [128, 4 * S], cdt, isOutput=False)
    wq_h = nc.declare_dram_parameter("wq", [128, 128], cdt, isOutput=False)
    wk_h = nc.declare_dram_parameter("wk", [128, 128], cdt, isOutput=False)
    wv_h = nc.declare_dram_parameter("wv", [128, 128], cdt, isOutput=False)
    mask_h = nc.declare_dram_parameter("mask8", [128, 1024], cdt, isOutput=False)
    ident_h = nc.declare_dram_parameter("ident", [128, 128], cdt, isOutput=False)
    ones_h = nc.declare_dram_parameter("ones", [1, 512], cdt, isOutput=False)
    zer_h = nc.declare_dram_parameter("zer", [1, 512], cdt, isOutput=False)
    out_h = nc.declare_dram_parameter("out", [B, S, 128], fp32, isOutput=True)
    den_h = nc.declare_dram_parameter("den", [B, S, 2], fp32, isOutput=True)

    NXT = 8               # number of x SBUF tiles
    XCOLS = 4 * S // NXT  # 2048 cols per tile = 4 chunks

    with TileContext(nc) as tc:
        with (
            tc.tile_pool(name="consts", bufs=1) as consts,
            tc.tile_pool(name="work", bufs=2) as work,
            # PSUM pools; creation order fixes bank layout (8 banks total)
            tc.tile_pool(name="pst", bufs=1, space="PSUM") as pst,
            tc.tile_pool(name="pu", bufs=1, space="PSUM") as pu,
            tc.tile_pool(name="pa", bufs=1, space="PSUM") as pa,
            tc.tile_pool(name="pnv", bufs=1, space="PSUM") as pnv,
            tc.tile_pool(name="pkn", bufs=1, space="PSUM") as pkn,
            tc.tile_pool(name="pdz", bufs=1, space="PSUM") as pdz,
        ):
            # ---- constants into SBUF ----
            wq = consts.tile([128, 128], cdt)
            wk = consts.tile([128, 128], cdt)
            wv = consts.tile([128, 128], cdt)
            mask8 = consts.tile([128, 1024], cdt)
            ident = consts.tile([128, 128], cdt)
            ones = consts.tile([1, 512], cdt)
            zer = consts.tile([1, 512], cdt)
            nc.sync.dma_start(wq, wq_h[:, :])
            nc.sync.dma_start(wk, wk_h[:, :])
            nc.sync.dma_start(wv, wv_h[:, :])
            nc.sync.dma_start(mask8, mask_h[:, :])
            nc.sync.dma_start(ident, ident_h[:, :])
            nc.sync.dma_start(ones, ones_h[:, :])
            nc.sync.dma_start(zer, zer_h[:, :])

            ones_col = consts.tile([128, 1], cdt)
            nc.gpsimd.memset(ones_col, 1.0)
            neg1 = consts.tile([128, 1], fp32)
            nc.gpsimd.memset(neg1, -1.0)

            xsb = []
            for t in range(NXT):
                xt = consts.tile([128, XCOLS], cdt, name=f"xsb{t}")
                nc.sync.dma_start(xt, xq_h[:, t * XCOLS:(t + 1) * XCOLS])
                xsb.append(xt)

            def xchunk(i):
                """[128, 512] x columns of chunk i (4 batches)."""
                t, r = divmod(i * 512, XCOLS)
                return xsb[t][:, r:r + 512]

            # ping-pong SBUF state copies for den path
            s01z = [consts.tile([128, 8], cdt, name=f"s01z{j}") for j in range(2)]
            for t in s01z:
                nc.gpsimd.memset(t, 0.0)
            zsum = [consts.tile([128, 4], fp32, name=f"zsum{j}") for j in range(2)]

            # ---- persistent state PSUM bank, zeroed once ----
            state = pst.tile([128, 512], fp32, name="state")
            nc.tensor.matmul(state, ones[:, 0:128], zer[:, 0:512],
                             start=True, stop=False, skip_group_check=True)

            def emit_front(i):
                """preset + q/k proj + v matmuls + exp/phi + vsb for chunk i.
                Returns (u, e2, phi2, vk, vsb)."""
                xc = xchunk(i)
                u = pu.tile([128, 1024], fp32, name="u")
                # +1 preset, one K=1 matmul per bank
                nc.tensor.matmul(u[:, 0:512], ones[:, 0:128], ones[:, 0:512],
                                 start=True, stop=False, skip_group_check=True)
                nc.tensor.matmul(u[:, 512:1024], ones[:, 0:128], ones[:, 0:512],
                                 start=True, stop=False, skip_group_check=True)
                nc.tensor.matmul(u[:, 0:512], wq, xc, start=False, stop=True,
                                 skip_group_check=True)
                nc.tensor.matmul(u[:, 512:1024], wk, xc, start=False, stop=True,
                                 skip_group_check=True)
                vk = pnv.tile([128, 512], fp32, name="vk", tag="nv")
                for b in range(4):
                    nc.tensor.matmul(vk[:, 128 * b:128 * (b + 1)],
                                     xc[:, 128 * b:128 * (b + 1)], wv,
                                     start=True, stop=True)
                # phi(u) = max(u, min(exp(u-1), 1)); u holds proj+1
                e2 = work.tile([128, 1024], cdt, name="e2")
                nc.scalar.activation(e2, u, Act.Exp, bias=neg1)
                phi2 = work.tile([128, 1024], cdt, name="phi2")
                nc.vector.scalar_tensor_tensor(phi2, e2, 1.0, u, Alu.min, Alu.max)
                vsb = work.tile([128, 512], cdt, name="vsb")
                nc.scalar.copy(vsb, vk)
                return u, e2, phi2, vk, vsb

            u, e2, phi2, vk, vsb = emit_front(0)
            s01v_prev = None

            for i in range(nchunk):
                sl = slice(i * C, (i + 1) * C)

                # ---- A^T = K Q^T per (b,h); h-major blocks [4xh0 | 4xh1] ----
                a_ps = pa.tile([128, 1024], fp32, name="a_ps")
                for h in range(2):
                    es = slice(64 * h, 64 * (h + 1))
                    for b in range(4):
                        nc.tensor.matmul(
                            a_ps[:, 512 * h + 128 * b:512 * h + 128 * (b + 1)],
                            phi2[es, 512 + 128 * b:512 + 128 * (b + 1)],
                            phi2[es, 128 * b:128 * (b + 1)],
                            start=True, stop=True)

                # ---- transpose phi(k) per batch -> token-major (bf16 PSUM) --
                knp = pkn.tile([128, 512], cdt, name="knp")
                for b in range(4):
                    nc.tensor.transpose(
                        knp[:, 128 * b:128 * (b + 1)],
                        phi2[:, 512 + 128 * b:512 + 128 * (b + 1)], ident)
                knat = work.tile([128, 512], cdt, name="knat")
                nc.scalar.copy(knat, knp)

                # ---- masked A -> SBUF (one wide DVE op) ----
                am2 = work.tile([128, 1024], cdt, name="am2")
                nc.vector.tensor_tensor(am2, a_ps, mask8, Alu.mult)

                num = pnv.tile([128, 512], fp32, name="num", tag="nv")
                denz = pdz.tile([128, 12], fp32, name="denz")

                # ---- cross-chunk terms: Q S_prev, Q z_prev ----
                if i > 0:
                    for b in range(4):
                        nc.tensor.matmul(
                            num[:, 128 * b:128 * (b + 1)],
                            phi2[:, 128 * b:128 * (b + 1)],
                            s01v_prev[:, 128 * b:128 * (b + 1)],
                            start=(b == 0), stop=False, skip_group_check=True)
                    for b in range(4):
                        nc.tensor.matmul(
                            denz[:, 2 * b:2 * b + 2],
                            phi2[:, 128 * b:128 * (b + 1)],
                            s01z[(i - 1) % 2][:, 2 * b:2 * b + 2],
                            start=(b == 0), stop=False, skip_group_check=True)

                # ---- next chunk front half (keeps PE/Act/DVE pipelined) ----
                if i + 1 < nchunk:
                    u, e2, phi2_n, vk, vsb_n = emit_front(i + 1)
                else:
                    phi2_n = vsb_n = None

                # ---- intra-chunk: num += am^T V, den += am^T 1 ----
                for h in range(2):
                    for b in range(4):
                        amb = am2[:, 512 * h + 128 * b:512 * h + 128 * (b + 1)]
                        nc.tensor.matmul(
                            num[:, 128 * b + 64 * h:128 * b + 64 * (h + 1)],
                            amb, vsb[:, 128 * b + 64 * h:128 * b + 64 * (h + 1)],
                            start=(i == 0 and h == 0 and b == 0), stop=True,
                            skip_group_check=True)
                        nc.tensor.matmul(
                            denz[:, 2 * b + h:2 * b + h + 1],
                            amb, ones_col,
                            start=(i == 0 and h == 0 and b == 0), stop=False,
                            skip_group_check=True)

                # ---- state update: S += Kt^T V, z_chunk = Kt^T 1 ----
                for h in range(2):
                    for b in range(4):
                        kt = knat[:, 128 * b + 64 * h:128 * b + 64 * (h + 1)]
                        nc.tensor.matmul(
                            state[64 * h:64 * (h + 1),
                                  128 * b + 64 * h:128 * b + 64 * (h + 1)],
                            kt, vsb[:, 128 * b + 64 * h:128 * b + 64 * (h + 1)],
                            start=False, stop=False, skip_group_check=True)
                        nc.tensor.matmul(
                            denz[64 * h:64 * (h + 1), 8 + b:9 + b],
                            kt, ones_col,
                            start=False, stop=(h == 1 and b == 3),
                            skip_group_check=True)

                # ---- state copies for next chunk's cross terms ----
                if i + 1 < nchunk:
                    s01v = work.tile([128, 512], cdt, name="s01v")
                    nc.scalar.copy(s01v, state)
                    s01v_prev = s01v
                    zc = denz[:, 8:12]
                    if i == 0:
                        nc.vector.tensor_copy(zsum[0], zc)
                    else:
                        nc.vector.tensor_tensor(zsum[i % 2], zc,
                                                zsum[(i - 1) % 2], Alu.add)
                    szt = s01z[i % 2]
                    szv = szt.rearrange("p (g c) -> p g c", c=2)
                    zs = zsum[i % 2]
                    nc.vector.tensor_copy(szv[0:64, :, 0:1], zs[0:64, :])
                    nc.vector.tensor_copy(szv[64:128, :, 1:2], zs[64:128, :])

                # ---- evacuate num/den and DMA out ----
                numout = work.tile([128, 512], fp32, name="numout")
                nc.scalar.copy(numout, num)
                denout = work.tile([128, 8], fp32, name="denout")
                nc.vector.tensor_copy(denout, denz[:, 0:8])
                for b in range(4):
                    nc.sync.dma_start(out_h[b, sl, :],
                                      numout[:, 128 * b:128 * (b + 1)])
                    nc.sync.dma_start(den_h[b, sl, :],
                                      denout[:, 2 * b:2 * b + 2])

                phi2 = phi2_n
                vsb = vsb_n

    nc.finalize()
    return nc


def _host_prep(x, Wq, Wk, Wv, Wp):
    """Shard inputs per core; returns in_maps list."""
    x = np.asarray(x, dtype=np.float32)
    Wq = np.asarray(Wq, dtype=np.float32)
    Wk = np.asarray(Wk, dtype=np.float32)
    Wv = np.asarray(Wv, dtype=np.float32)
    Wp = np.asarray(Wp, dtype=np.float32)
    ndt = ml_dtypes.bfloat16

    mask = np.triu(np.ones((C, C), np.float32))
    mask8 = np.tile(mask, (1, 8)).astype(ndt)          # [128, 1024]
    ident = np.eye(128, dtype=np.float32).astype(ndt)
    ones = np.ones((1, 512), np.float32).astype(ndt)
    zer = np.zeros((1, 512), np.float32).astype(ndt)

    in_maps = []
    for c in range(NCORE):
        h0 = HPC * c
        xs = x[:, :, 64 * h0:64 * (h0 + HPC)]          # [B, S, 128]
        xT = xs.transpose(2, 0, 1)                     # [128, B, S]
        # col = 512*chunk + 128*b + s_in_chunk
        xq = np.ascontiguousarray(
            xT.reshape(128, B, NCHUNK, C).transpose(0, 2, 1, 3)
        ).reshape(128, B * S).astype(ndt)
        wq_bd = np.zeros((128, 128), np.float32)
        wk_bd = np.zeros((128, 128), np.float32)
        wv_bd = np.zeros((128, 128), np.float32)
        for j in range(HPC):
            h = h0 + j
            sl = slice(64 * j, 64 * (j + 1))
            wq_bd[sl, sl] = Wq[h]
            wk_bd[sl, sl] = Wk[h]
            wv_bd[sl, sl] = Wv[h] @ Wp[64 * h:64 * (h + 1), :]
        in_maps.append({
            "xq": xq,
            "wq": wq_bd.astype(ndt),
            "wk": wk_bd.astype(ndt),
            "wv": wv_bd.astype(ndt),
            "mask8": mask8,
            "ident": ident,
            "ones": ones,
            "zer": zer,
        })
    return in_maps


def get_program():
    if "nc" not in _CACHE:
        _CACHE["nc"] = _build_program()
    return _CACHE["nc"]


def run_spmd(in_maps, **kwargs):
    from concourse.bass_utils import run_bass_kernel_spmd
    nc = get_program()
    return run_bass_kernel_spmd(nc, in_maps, list(range(NCORE)), **kwargs)


def kernel(x, Wq, Wk, Wv, Wp):
    in_maps = _host_prep(x, Wq, Wk, Wv, Wp)
    res = run_spmd(in_maps)
    out = np.zeros((B, S, O), np.float32)
    for c in range(NCORE):
        num = res.results[c]["out"]                    # [B, S, 128]
        den = res.results[c]["den"]                    # [B, S, 2]
        out += num[:, :, 0:64] / (den[:, :, 0:1] + EPS)
        out += num[:, :, 64:128] / (den[:, :, 1:2] + EPS)
    return out


# revision 5
# speedup vs baseline: 1.0051x; 1.0051x over previous
"""Trainium2 Bass kernel for MultiLinearAttention (causal linear attention).

Reference computation (per head h, feature map phi(u) = elu(u)+1):
    q = phi(x_h @ Wq_h), k = phi(x_h @ Wk_h), v = x_h @ Wv_h
    y_t = (q_t . sum_{s<=t} k_s v_s^T) / (q_t . sum_{s<=t} k_s + eps)
    out = concat_h(y_h) @ Wp

Sharding: 16 heads / 8 cores = 2 heads per core, all 4 batches per core.
Wp is folded per-head into the v projection (W'_h = Wv_h @ Wp_h); each core
ships per-head numerators [B, S, 2, 64] and denominators [B, S, 2]; the
host unshard computes y = sum_cores sum_h num_h / (den_h + eps).

Device algorithm: chunked causal linear attention, chunk C=128, all 4
batches fused per chunk into wide ops:
    u = 1 + [q|k] projections (PSUM preset via K=1 ones matmul)
    phi = max(u, min(exp(u-1), 1)) == elu(.)+1
    A^T = K Q^T per (b,h) (8 blocks, h-major: [4xh0 | 4xh1])
    am = A ⊙ causal-mask (one DVE op over all 8 blocks)
    num = am^T V + Q S_prev; den = am^T 1 + Q z_prev
    S += Kt^T V (PSUM-persistent); z via PSUM chunk-sums + SBUF f32 acc.

PSUM banks (8): state[512]f32 | u[1024]f32 x2 | A[1024]f32 x2 |
num[512]f32 | vk[512]f32 | {knp[512]bf16 + den/z [12]f32} shared.
"""

import os
import sys

import numpy as np

for _p in ("/root/.axon_site/_ro/trn_rl_repo", "/opt/trn_rl_repo", "/opt/pypackages"):
    if os.path.isdir(_p) and _p not in sys.path:
        sys.path.append(_p)

import ml_dtypes

B, S, D = 4, 4096, 1024
H, HD, O = 16, 64, 64
C = 128                  # chunk length
NCORE = 8
HPC = H // NCORE         # heads per core
NCHUNK = S // C
EPS = 1e-6

_CACHE = {}


def _build_program(nchunk=NCHUNK):
    import concourse.mybir as mybir
    from concourse import bacc
    from concourse.tile import TileContext

    fp32 = mybir.dt.float32
    cdt = mybir.dt.bfloat16
    Alu = mybir.AluOpType
    Act = mybir.ActivationFunctionType

    nc = bacc.Bacc()
    # x staged feature-major, chunk-interleaved: col = 512*chunk + 128*b + s
    xq_h = nc.declare_dram_parameter("xq", [128, 4 * S], cdt, isOutput=False)
    wq_h = nc.declare_dram_parameter("wq", [128, 128], cdt, isOutput=False)
    wk_h = nc.declare_dram_parameter("wk", [128, 128], cdt, isOutput=False)
    wv_h = nc.declare_dram_parameter("wv", [128, 128], cdt, isOutput=False)
    mask_h = nc.declare_dram_parameter("mask8", [128, 1024], cdt, isOutput=False)
    ident_h = nc.declare_dram_parameter("ident", [128, 128], cdt, isOutput=False)
    ones_h = nc.declare_dram_parameter("ones", [1, 512], cdt, isOutput=False)
    zer_h = nc.declare_dram_parameter("zer", [1, 512], cdt, isOutput=False)
    out_h = nc.declare_dram_parameter("out", [B, S, 128], fp32, isOutput=True)
    den_h = nc.declare_dram_parameter("den", [B, S, 2], fp32, isOutput=True)

    NXT = 8               # number of x SBUF tiles
    XCOLS = 4 * S // NXT  # 2048 cols per tile = 4 chunks

    with TileContext(nc) as tc:
        with (
            tc.tile_pool(name="consts", bufs=1) as consts,
            tc.tile_pool(name="work", bufs=2) as work,
            # PSUM pools; creation order fixes bank layout (8 banks total)
            tc.tile_pool(name="pst", bufs=1, space="PSUM") as pst,
            tc.tile_pool(name="pu", bufs=1, space="PSUM") as pu,
            tc.tile_pool(name="pa", bufs=1, space="PSUM") as pa,
            tc.tile_pool(name="pn", bufs=1, space="PSUM") as pn,
            tc.tile_pool(name="pv", bufs=1, space="PSUM") as pv,
            tc.tile_pool(name="pkz", bufs=1, space="PSUM") as pkz,
        ):
            # ---- constants into SBUF ----
            wq = consts.tile([128, 128], cdt)
            wk = consts.tile([128, 128], cdt)
            wv = consts.tile([128, 128], cdt)
            mask8 = consts.tile([128, 1024], cdt)
            ident = consts.tile([128, 128], cdt)
            ones = consts.tile([1, 512], cdt)
            zer = consts.tile([1, 512], cdt)
            nc.sync.dma_start(wq, wq_h[:, :])
            nc.sync.dma_start(wk, wk_h[:, :])
            nc.sync.dma_start(wv, wv_h[:, :])
            nc.sync.dma_start(mask8, mask_h[:, :])
            nc.sync.dma_start(ident, ident_h[:, :])
            nc.sync.dma_start(ones, ones_h[:, :])
            nc.sync.dma_start(zer, zer_h[:, :])

            ones_col = consts.tile([128, 1], cdt)
            nc.gpsimd.memset(ones_col, 1.0)
            neg1 = consts.tile([128, 1], fp32)
            nc.gpsimd.memset(neg1, -1.0)

            xsb = []
            for t in range(NXT):
                xt = consts.tile([128, XCOLS], cdt, name=f"xsb{t}")
                nc.sync.dma_start(xt, xq_h[:, t * XCOLS:(t + 1) * XCOLS])
                xsb.append(xt)

            def xchunk(i):
                """[128, 512] x columns of chunk i (4 batches)."""
                t, r = divmod(i * 512, XCOLS)
                return xsb[t][:, r:r + 512]

            # ping-pong SBUF state copies for den path
            s01z = [consts.tile([128, 8], cdt, name=f"s01z{j}") for j in range(2)]
            for t in s01z:
                nc.gpsimd.memset(t, 0.0)
            zsum = [consts.tile([128, 4], fp32, name=f"zsum{j}") for j in range(2)]

            # ---- persistent state PSUM bank, zeroed once ----
            state = pst.tile([128, 512], fp32, name="state")
            nc.tensor.matmul(state, ones[:, 0:128], zer[:, 0:512],
                             start=True, stop=False, skip_group_check=True)

            def emit_uproj(i):
                """preset + q/k projections for chunk i -> u tile."""
                xc = xchunk(i)
                u = pu.tile([128, 1024], fp32, name="u")
                nc.tensor.matmul(u[:, 0:512], ones[:, 0:128], ones[:, 0:512],
                                 start=True, stop=False, skip_group_check=True)
                nc.tensor.matmul(u[:, 512:1024], ones[:, 0:128], ones[:, 0:512],
                                 start=True, stop=False, skip_group_check=True)
                nc.tensor.matmul(u[:, 0:512], wq, xc, start=False, stop=True,
                                 skip_group_check=True)
                nc.tensor.matmul(u[:, 512:1024], wk, xc, start=False, stop=True,
                                 skip_group_check=True)
                return u

            def emit_v(i):
                xc = xchunk(i)
                vk = pv.tile([128, 512], fp32, name="vk")
                for b in range(4):
                    nc.tensor.matmul(vk[:, 128 * b:128 * (b + 1)],
                                     xc[:, 128 * b:128 * (b + 1)], wv,
                                     start=(b == 0), stop=(b == 3),
                                     skip_group_check=True)
                return vk

            def emit_phi(u):
                """phi(u) = max(u, min(exp(u-1), 1)); u holds proj+1."""
                e2 = work.tile([128, 1024], cdt, name="e2")
                nc.scalar.activation(e2, u, Act.Exp, bias=neg1)
                phi2 = work.tile([128, 1024], cdt, name="phi2")
                nc.vector.scalar_tensor_tensor(phi2, e2, 1.0, u, Alu.min, Alu.max)
                return phi2

            # ---- prologue: chunk 0 front ----
            u = emit_uproj(0)
            vk = emit_v(0)
            phi2 = emit_phi(u)
            vsb = work.tile([128, 512], cdt, name="vsb")
            nc.scalar.copy(vsb, vk)
            s01v_prev = None

            for i in range(nchunk):
                sl = slice(i * C, (i + 1) * C)

                # ---- A^T = K Q^T per (b,h); h-major blocks [4xh0 | 4xh1] ----
                a_ps = pa.tile([128, 1024], fp32, name="a_ps")
                for h in range(2):
                    es = slice(64 * h, 64 * (h + 1))
                    for b in range(4):
                        nc.tensor.matmul(
                            a_ps[:, 512 * h + 128 * b:512 * h + 128 * (b + 1)],
                            phi2[es, 512 + 128 * b:512 + 128 * (b + 1)],
                            phi2[es, 128 * b:128 * (b + 1)],
                            start=True, stop=True)
                # masked A -> SBUF (one wide DVE op)
                am2 = work.tile([128, 1024], cdt, name="am2")
                nc.vector.tensor_tensor(am2, a_ps, mask8, Alu.mult)

                # ---- next-chunk projections early: shortens exp->phi chain --
                if i + 1 < nchunk:
                    u = emit_uproj(i + 1)
                    vk = emit_v(i + 1)

                # ---- transpose phi(k) per batch -> token-major (bf16 PSUM) --
                # knp shares its bank with den/z; transposes must precede the
                # den writers of this chunk (PE order does that).
                kdz = pkz.tile([128, 544], cdt, name="kdz")
                knp = kdz[:, 0:512]
                denz = kdz[:, 512:536].bitcast(fp32)   # [128, 12] f32
                for b in range(4):
                    nc.tensor.transpose(
                        knp[:, 128 * b:128 * (b + 1)],
                        phi2[:, 512 + 128 * b:512 + 128 * (b + 1)], ident)
                knat = work.tile([128, 512], cdt, name="knat")
                nc.vector.tensor_copy(knat.bitcast(fp32), knp.bitcast(fp32))

                num = pn.tile([128, 512], fp32, name="num")

                # ---- cross-chunk terms: Q S_prev, Q z_prev ----
                if i > 0:
                    for b in range(4):
                        nc.tensor.matmul(
                            num[:, 128 * b:128 * (b + 1)],
                            phi2[:, 128 * b:128 * (b + 1)],
                            s01v_prev[:, 128 * b:128 * (b + 1)],
                            start=(b == 0), stop=False, skip_group_check=True)
                    for b in range(4):
                        nc.tensor.matmul(
                            denz[:, 2 * b:2 * b + 2],
                            phi2[:, 128 * b:128 * (b + 1)],
                            s01z[(i - 1) % 2][:, 2 * b:2 * b + 2],
                            start=(b == 0), stop=False, skip_group_check=True)

                # exp/phi for next chunk (Act+DVE overlap with PE below)
                if i + 1 < nchunk:
                    phi2_n = emit_phi(u)
                else:
                    phi2_n = None

                # ---- intra-chunk: num += am^T V, den += am^T 1 ----
                for h in range(2):
                    for b in range(4):
                        amb = am2[:, 512 * h + 128 * b:512 * h + 128 * (b + 1)]
                        nc.tensor.matmul(
                            num[:, 128 * b + 64 * h:128 * b + 64 * (h + 1)],
                            amb, vsb[:, 128 * b + 64 * h:128 * b + 64 * (h + 1)],
                            start=(i == 0 and h == 0 and b == 0), stop=True,
                            skip_group_check=True)
                        nc.tensor.matmul(
                            denz[:, 2 * b + h:2 * b + h + 1],
                            amb, ones_col,
                            start=(i == 0 and h == 0 and b == 0), stop=False,
                            skip_group_check=True)

                # ---- state update: S += Kt^T V; z_chunk = Kt^T 1 ----
                for h in range(2):
                    for b in range(4):
                        kt = knat[:, 128 * b + 64 * h:128 * b + 64 * (h + 1)]
                        nc.tensor.matmul(
                            state[64 * h:64 * (h + 1),
                                  128 * b + 64 * h:128 * b + 64 * (h + 1)],
                            kt, vsb[:, 128 * b + 64 * h:128 * b + 64 * (h + 1)],
                            start=False, stop=False, skip_group_check=True)
                for b in range(4):
                    nc.tensor.matmul(
                        denz[:, 8 + b:9 + b],
                        knat[:, 128 * b:128 * (b + 1)], ones_col,
                        start=False, stop=(b == 3), skip_group_check=True)

                # ---- state copies for next chunk's cross terms ----
                if i + 1 < nchunk:
                    s01v = work.tile([128, 512], cdt, name="s01v")
                    nc.scalar.copy(s01v, state)
                    s01v_prev = s01v
                    zc = denz[:, 8:12]
                    if i == 0:
                        nc.vector.tensor_copy(zsum[0], zc)
                    else:
                        nc.vector.tensor_tensor(zsum[i % 2], zc,
                                                zsum[(i - 1) % 2], Alu.add)
                    szt = s01z[i % 2]
                    szv = szt.rearrange("p (g c) -> p g c", c=2)
                    zs = zsum[i % 2]
                    nc.vector.tensor_copy(szv[0:64, :, 0:1], zs[0:64, :])
                    nc.vector.tensor_copy(szv[64:128, :, 1:2], zs[64:128, :])
                    # vsb for next chunk
                    vsb_n = work.tile([128, 512], cdt, name="vsb")
                    nc.scalar.copy(vsb_n, vk)
                else:
                    vsb_n = None

                # ---- evacuate num/den and DMA out ----
                numout = work.tile([128, 512], fp32, name="numout")
                nc.scalar.copy(numout, num)
                denout = work.tile([128, 8], fp32, name="denout")
                nc.vector.tensor_copy(denout, denz[:, 0:8])
                for b in range(4):
                    nc.sync.dma_start(out_h[b, sl, :],
                                      numout[:, 128 * b:128 * (b + 1)])
                    nc.sync.dma_start(den_h[b, sl, :],
                                      denout[:, 2 * b:2 * b + 2])

                phi2 = phi2_n
                vsb = vsb_n

    nc.finalize()
    return nc


def _host_prep(x, Wq, Wk, Wv, Wp):
    """Shard inputs per core; returns in_maps list."""
    x = np.asarray(x, dtype=np.float32)
    Wq = np.asarray(Wq, dtype=np.float32)
    Wk = np.asarray(Wk, dtype=np.float32)
    Wv = np.asarray(Wv, dtype=np.float32)
    Wp = np.asarray(Wp, dtype=np.float32)
    ndt = ml_dtypes.bfloat16

    mask = np.triu(np.ones((C, C), np.float32))
    mask8 = np.tile(mask, (1, 8)).astype(ndt)          # [128, 1024]
    ident = np.eye(128, dtype=np.float32).astype(ndt)
    ones = np.ones((1, 512), np.float32).astype(ndt)
    zer = np.zeros((1, 512), np.float32).astype(ndt)

    in_maps = []
    for c in range(NCORE):
        h0 = HPC * c
        xs = x[:, :, 64 * h0:64 * (h0 + HPC)]          # [B, S, 128]
        xT = xs.transpose(2, 0, 1)                     # [128, B, S]
        # col = 512*chunk + 128*b + s_in_chunk
        xq = np.ascontiguousarray(
            xT.reshape(128, B, NCHUNK, C).transpose(0, 2, 1, 3)
        ).reshape(128, B * S).astype(ndt)
        wq_bd = np.zeros((128, 128), np.float32)
        wk_bd = np.zeros((128, 128), np.float32)
        wv_bd = np.zeros((128, 128), np.float32)
        for j in range(HPC):
            h = h0 + j
            sl = slice(64 * j, 64 * (j + 1))
            wq_bd[sl, sl] = Wq[h]
            wk_bd[sl, sl] = Wk[h]
            wv_bd[sl, sl] = Wv[h] @ Wp[64 * h:64 * (h + 1), :]
        in_maps.append({
            "xq": xq,
            "wq": wq_bd.astype(ndt),
            "wk": wk_bd.astype(ndt),
            "wv": wv_bd.astype(ndt),
            "mask8": mask8,
            "ident": ident,
            "ones": ones,
            "zer": zer,
        })
    return in_maps


def get_program():
    if "nc" not in _CACHE:
        _CACHE["nc"] = _build_program()
    return _CACHE["nc"]


def run_spmd(in_maps, **kwargs):
    from concourse.bass_utils import run_bass_kernel_spmd
    nc = get_program()
    return run_bass_kernel_spmd(nc, in_maps, list(range(NCORE)), **kwargs)


def kernel(x, Wq, Wk, Wv, Wp):
    in_maps = _host_prep(x, Wq, Wk, Wv, Wp)
    res = run_spmd(in_maps)
    out = np.zeros((B, S, O), np.float32)
    for c in range(NCORE):
        num = res.results[c]["out"]                    # [B, S, 128]
        den = res.results[c]["den"]                    # [B, S, 2]
        out += num[:, :, 0:64] / (den[:, :, 0:1] + EPS)
        out += num[:, :, 64:128] / (den[:, :, 1:2] + EPS)
    return out


# revision 10
# speedup vs baseline: 1.5648x; 1.5568x over previous
"""Trainium2 Bass kernel for MultiLinearAttention (causal linear attention).

Reference computation (per head h, feature map phi(u) = elu(u)+1):
    q = phi(x_h @ Wq_h), k = phi(x_h @ Wk_h), v = x_h @ Wv_h
    y_t = (q_t . sum_{s<=t} k_s v_s^T) / (q_t . sum_{s<=t} k_s + eps)
    out = concat_h(y_h) @ Wp

Sharding: 16 heads / 8 cores = 2 heads per core, all 4 batches per core.
Wp is folded per-head into the v projection (W'_h = Wv_h @ Wp_h); each core
ships per-head numerators [B, S, 2, 64] and denominators [B, S, 2]; the
host unshard computes y = sum_cores sum_h num_h / (den_h + eps).

Device algorithm: chunked causal linear attention, chunk C=128, all 4
batches fused per chunk into wide ops:
    u = 1 + [q|k] projections (PSUM preset via K=1 ones matmul)
    phi = max(u, min(exp(u-1), 1)) == elu(.)+1
    A^T = K Q^T per (b,h) (8 blocks, h-major: [4xh0 | 4xh1])
    am = A ⊙ causal-mask (one DVE op over all 8 blocks)
    num = am^T V + Q S_prev; den = am^T 1 + Q z_prev
    S += Kt^T V (PSUM-persistent); z via PSUM chunk-sums + SBUF f32 acc.

PSUM banks (8): state[512]f32 | u[1024]f32 x2 | A[1024]f32 x2 |
num[512]f32 | vk[512]f32 | {knp[512]bf16 + den/z [12]f32} shared.
"""

import os
import sys

import numpy as np

for _p in ("/root/.axon_site/_ro/trn_rl_repo", "/opt/trn_rl_repo", "/opt/pypackages"):
    if os.path.isdir(_p) and _p not in sys.path:
        sys.path.append(_p)

import ml_dtypes

B, S, D = 4, 4096, 1024
H, HD, O = 16, 64, 64
C = 128                  # chunk length
NCORE = 8
HPC = H // NCORE         # heads per core
NCHUNK = S // C
EPS = 1e-6

_CACHE = {}


def _build_program(nchunk=NCHUNK):
    import concourse.mybir as mybir
    from concourse import bacc
    from concourse.tile import TileContext

    fp32 = mybir.dt.float32
    cdt = mybir.dt.bfloat16
    Alu = mybir.AluOpType
    Act = mybir.ActivationFunctionType

    nc = bacc.Bacc()
    # x staged feature-major, chunk-interleaved: col = 512*chunk + 128*b + s
    xq_h = nc.declare_dram_parameter("xq", [128, 4 * S], cdt, isOutput=False)
    wq_h = nc.declare_dram_parameter("wq", [128, 128], cdt, isOutput=False)
    wk_h = nc.declare_dram_parameter("wk", [128, 128], cdt, isOutput=False)
    wv_h = nc.declare_dram_parameter("wv", [128, 128], cdt, isOutput=False)
    mask_h = nc.declare_dram_parameter("mask8", [128, 1024], cdt, isOutput=False)
    ident_h = nc.declare_dram_parameter("ident", [128, 128], cdt, isOutput=False)
    ones_h = nc.declare_dram_parameter("ones", [1, 512], cdt, isOutput=False)
    zer_h = nc.declare_dram_parameter("zer", [1, 512], cdt, isOutput=False)
    out_h = nc.declare_dram_parameter("out", [B, S, 128], fp32, isOutput=True)
    den_h = nc.declare_dram_parameter("den", [B, S, 2], fp32, isOutput=True)

    NXT = 8               # number of x SBUF tiles
    XCOLS = 4 * S // NXT  # 2048 cols per tile = 4 chunks

    with TileContext(nc) as tc:
        with (
            tc.tile_pool(name="consts", bufs=1) as consts,
            tc.tile_pool(name="work", bufs=2) as work,
            tc.tile_pool(name="stage", bufs=2) as stage,
            # PSUM pools; creation order fixes bank layout (8 banks total)
            tc.tile_pool(name="pst", bufs=1, space="PSUM") as pst,
            tc.tile_pool(name="pu", bufs=1, space="PSUM") as pu,
            tc.tile_pool(name="pa", bufs=1, space="PSUM") as pa,
            tc.tile_pool(name="pn", bufs=1, space="PSUM") as pn,
            tc.tile_pool(name="pv", bufs=1, space="PSUM") as pv,
            tc.tile_pool(name="pkz", bufs=1, space="PSUM") as pkz,
        ):
            # ---- constants into SBUF ----
            wq = consts.tile([128, 128], cdt)
            wk = consts.tile([128, 128], cdt)
            wv = consts.tile([128, 128], cdt)
            mask8 = consts.tile([128, 1024], cdt)
            ident = consts.tile([128, 128], cdt)
            ones = consts.tile([1, 512], cdt)
            zer = consts.tile([1, 512], cdt)
            # SWDGE (gpsimd) for input loads keeps the SP sequencer free for
            # the batched output DMAs.
            nc.gpsimd.dma_start(wq, wq_h[:, :])
            nc.gpsimd.dma_start(wk, wk_h[:, :])
            nc.gpsimd.dma_start(wv, wv_h[:, :])
            nc.gpsimd.dma_start(mask8, mask_h[:, :])
            nc.gpsimd.dma_start(ident, ident_h[:, :])
            nc.gpsimd.dma_start(ones, ones_h[:, :])
            nc.gpsimd.dma_start(zer, zer_h[:, :])

            ones_col = consts.tile([128, 1], cdt)
            nc.gpsimd.memset(ones_col, 1.0)
            neg1 = consts.tile([128, 1], fp32)
            nc.gpsimd.memset(neg1, -1.0)

            xsb = []
            for t in range(NXT):
                xt = consts.tile([128, XCOLS], cdt, name=f"xsb{t}")
                nc.gpsimd.dma_start(xt, xq_h[:, t * XCOLS:(t + 1) * XCOLS])
                xsb.append(xt)

            def xchunk(i):
                """[128, 512] x columns of chunk i (4 batches)."""
                t, r = divmod(i * 512, XCOLS)
                return xsb[t][:, r:r + 512]

            # ping-pong SBUF state copies for den path
            s01z = [consts.tile([128, 8], cdt, name=f"s01z{j}") for j in range(2)]
            for t in s01z:
                nc.gpsimd.memset(t, 0.0)
            zsum = [consts.tile([128, 4], fp32, name=f"zsum{j}") for j in range(2)]

            # ---- persistent state PSUM bank, zeroed once ----
            state = pst.tile([128, 512], fp32, name="state")
            nc.tensor.matmul(state, ones[:, 0:128], zer[:, 0:512],
                             start=True, stop=False, skip_group_check=True)

            def emit_uproj(i):
                """preset + q/k projections for chunk i -> u tile."""
                xc = xchunk(i)
                u = pu.tile([128, 1024], fp32, name="u")
                nc.tensor.matmul(u[:, 0:512], ones[:, 0:128], ones[:, 0:512],
                                 start=True, stop=False, skip_group_check=True)
                nc.tensor.matmul(u[:, 512:1024], ones[:, 0:128], ones[:, 0:512],
                                 start=True, stop=False, skip_group_check=True)
                nc.tensor.matmul(u[:, 0:512], wq, xc, start=False, stop=True,
                                 skip_group_check=True)
                nc.tensor.matmul(u[:, 512:1024], wk, xc, start=False, stop=True,
                                 skip_group_check=True)
                return u

            def emit_v(i):
                xc = xchunk(i)
                vk = pv.tile([128, 512], fp32, name="vk")
                for b in range(4):
                    nc.tensor.matmul(vk[:, 128 * b:128 * (b + 1)],
                                     xc[:, 128 * b:128 * (b + 1)], wv,
                                     start=(b == 0), stop=(b == 3),
                                     skip_group_check=True)
                return vk

            def emit_phi(u):
                """phi(u) = max(u, min(exp(u-1), 1)); u holds proj+1."""
                e2 = work.tile([128, 1024], cdt, name="e2")
                nc.scalar.activation(e2, u, Act.Exp, bias=neg1)
                phi2 = work.tile([128, 1024], cdt, name="phi2")
                nc.vector.scalar_tensor_tensor(phi2, e2, 1.0, u, Alu.min, Alu.max)
                return phi2

            # ---- prologue: chunk 0 front ----
            u = emit_uproj(0)
            vk = emit_v(0)
            phi2 = emit_phi(u)
            vsb = work.tile([128, 512], cdt, name="vsb")
            nc.scalar.copy(vsb, vk)
            s01v_prev = None
            numwide = denwide = None
            BCH = 8               # chunks per output-DMA batch

            for i in range(nchunk):
                if i % BCH == 0:
                    numwide = stage.tile([128, 512 * BCH], fp32, name="numwide")
                    denwide = stage.tile([128, 8 * BCH], fp32, name="denwide")

                # ---- A^T = K Q^T per (b,h); h-major blocks [4xh0 | 4xh1] ----
                a_ps = pa.tile([128, 1024], fp32, name="a_ps")
                for h in range(2):
                    es = slice(64 * h, 64 * (h + 1))
                    for b in range(4):
                        nc.tensor.matmul(
                            a_ps[:, 512 * h + 128 * b:512 * h + 128 * (b + 1)],
                            phi2[es, 512 + 128 * b:512 + 128 * (b + 1)],
                            phi2[es, 128 * b:128 * (b + 1)],
                            start=True, stop=True)
                # masked A -> SBUF (one wide DVE op)
                am2 = work.tile([128, 1024], cdt, name="am2")
                nc.vector.tensor_tensor(am2, a_ps, mask8, Alu.mult)

                # ---- next-chunk projections early: shortens exp->phi chain --
                if i + 1 < nchunk:
                    u = emit_uproj(i + 1)
                    vk = emit_v(i + 1)

                # ---- transpose phi(k) per batch -> token-major (bf16 PSUM) --
                # knp shares its bank with den/z; transposes must precede the
                # den writers of this chunk (PE order does that).
                kdz = pkz.tile([128, 544], cdt, name="kdz")
                knp = kdz[:, 0:512]
                denz = kdz[:, 512:536].bitcast(fp32)   # [128, 12] f32
                for b in range(4):
                    nc.tensor.transpose(
                        knp[:, 128 * b:128 * (b + 1)],
                        phi2[:, 512 + 128 * b:512 + 128 * (b + 1)], ident)
                knat = work.tile([128, 512], cdt, name="knat")
                nc.vector.tensor_copy(knat.bitcast(fp32), knp.bitcast(fp32))

                num = pn.tile([128, 512], fp32, name="num")

                # ---- cross-chunk terms: Q S_prev, Q z_prev ----
                if i > 0:
                    for b in range(4):
                        nc.tensor.matmul(
                            num[:, 128 * b:128 * (b + 1)],
                            phi2[:, 128 * b:128 * (b + 1)],
                            s01v_prev[:, 128 * b:128 * (b + 1)],
                            start=(b == 0), stop=False, skip_group_check=True)
                    for b in range(4):
                        nc.tensor.matmul(
                            denz[:, 2 * b:2 * b + 2],
                            phi2[:, 128 * b:128 * (b + 1)],
                            s01z[(i - 1) % 2][:, 2 * b:2 * b + 2],
                            start=(b == 0), stop=False, skip_group_check=True)

                # exp/phi for next chunk (Act+DVE overlap with PE below)
                if i + 1 < nchunk:
                    phi2_n = emit_phi(u)
                else:
                    phi2_n = None

                # ---- intra-chunk: num += am^T V, den += am^T 1 ----
                for h in range(2):
                    for b in range(4):
                        amb = am2[:, 512 * h + 128 * b:512 * h + 128 * (b + 1)]
                        nc.tensor.matmul(
                            num[:, 128 * b + 64 * h:128 * b + 64 * (h + 1)],
                            amb, vsb[:, 128 * b + 64 * h:128 * b + 64 * (h + 1)],
                            start=(i == 0 and h == 0 and b == 0), stop=True,
                            skip_group_check=True)
                        nc.tensor.matmul(
                            denz[:, 2 * b + h:2 * b + h + 1],
                            amb, ones_col,
                            start=(i == 0 and h == 0 and b == 0), stop=False,
                            skip_group_check=True)

                # ---- state update: S += Kt^T V; z_chunk = Kt^T 1 ----
                for h in range(2):
                    for b in range(4):
                        kt = knat[:, 128 * b + 64 * h:128 * b + 64 * (h + 1)]
                        nc.tensor.matmul(
                            state[64 * h:64 * (h + 1),
                                  128 * b + 64 * h:128 * b + 64 * (h + 1)],
                            kt, vsb[:, 128 * b + 64 * h:128 * b + 64 * (h + 1)],
                            start=False, stop=False, skip_group_check=True)
                for b in range(4):
                    nc.tensor.matmul(
                        denz[:, 8 + b:9 + b],
                        knat[:, 128 * b:128 * (b + 1)], ones_col,
                        start=False, stop=(b == 3), skip_group_check=True)

                # ---- state copies for next chunk's cross terms ----
                if i + 1 < nchunk:
                    s01v = work.tile([128, 512], cdt, name="s01v")
                    nc.scalar.copy(s01v, state)
                    s01v_prev = s01v
                    zc = denz[:, 8:12]
                    if i == 0:
                        nc.vector.tensor_copy(zsum[0], zc)
                    else:
                        nc.vector.tensor_tensor(zsum[i % 2], zc,
                                                zsum[(i - 1) % 2], Alu.add)
                    szt = s01z[i % 2]
                    szv = szt.rearrange("p (g c) -> p g c", c=2)
                    zs = zsum[i % 2]
                    nc.vector.tensor_copy(szv[0:64, :, 0:1], zs[0:64, :])
                    nc.vector.tensor_copy(szv[64:128, :, 1:2], zs[64:128, :])
                    # vsb for next chunk
                    vsb_n = work.tile([128, 512], cdt, name="vsb")
                    nc.scalar.copy(vsb_n, vk)
                else:
                    vsb_n = None

                # ---- evacuate num/den into wide staging; DMA every BCH ----
                ci = i % BCH
                nc.scalar.copy(numwide[:, 512 * ci:512 * (ci + 1)], num)
                nc.vector.tensor_copy(denwide[:, 8 * ci:8 * (ci + 1)],
                                      denz[:, 0:8])
                if ci == BCH - 1:
                    blk = slice((i - ci) * C, (i + 1) * C)
                    nwv = numwide.rearrange("p (c bo) -> p c bo", bo=512)
                    dwv = denwide.rearrange("p (c d) -> p c d", d=8)
                    for b in range(4):
                        nc.sync.dma_start(
                            out_h[b, blk, :].rearrange("(c s) o -> s c o",
                                                       s=128),
                            nwv[:, :, 128 * b:128 * (b + 1)])
                        nc.sync.dma_start(
                            den_h[b, blk, :].rearrange("(c s) d -> s c d",
                                                       s=128),
                            dwv[:, :, 2 * b:2 * b + 2])

                phi2 = phi2_n
                vsb = vsb_n

    nc.finalize()
    return nc


def _host_prep(x, Wq, Wk, Wv, Wp):
    """Shard inputs per core; returns in_maps list."""
    x = np.asarray(x, dtype=np.float32)
    Wq = np.asarray(Wq, dtype=np.float32)
    Wk = np.asarray(Wk, dtype=np.float32)
    Wv = np.asarray(Wv, dtype=np.float32)
    Wp = np.asarray(Wp, dtype=np.float32)
    ndt = ml_dtypes.bfloat16

    mask = np.triu(np.ones((C, C), np.float32))
    mask8 = np.tile(mask, (1, 8)).astype(ndt)          # [128, 1024]
    ident = np.eye(128, dtype=np.float32).astype(ndt)
    ones = np.ones((1, 512), np.float32).astype(ndt)
    zer = np.zeros((1, 512), np.float32).astype(ndt)

    in_maps = []
    for c in range(NCORE):
        h0 = HPC * c
        xs = x[:, :, 64 * h0:64 * (h0 + HPC)]          # [B, S, 128]
        xT = xs.transpose(2, 0, 1)                     # [128, B, S]
        # col = 512*chunk + 128*b + s_in_chunk
        xq = np.ascontiguousarray(
            xT.reshape(128, B, NCHUNK, C).transpose(0, 2, 1, 3)
        ).reshape(128, B * S).astype(ndt)
        wq_bd = np.zeros((128, 128), np.float32)
        wk_bd = np.zeros((128, 128), np.float32)
        wv_bd = np.zeros((128, 128), np.float32)
        for j in range(HPC):
            h = h0 + j
            sl = slice(64 * j, 64 * (j + 1))
            wq_bd[sl, sl] = Wq[h]
            wk_bd[sl, sl] = Wk[h]
            wv_bd[sl, sl] = Wv[h] @ Wp[64 * h:64 * (h + 1), :]
        in_maps.append({
            "xq": xq,
            "wq": wq_bd.astype(ndt),
            "wk": wk_bd.astype(ndt),
            "wv": wv_bd.astype(ndt),
            "mask8": mask8,
            "ident": ident,
            "ones": ones,
            "zer": zer,
        })
    return in_maps


def get_program():
    if "nc" not in _CACHE:
        _CACHE["nc"] = _build_program()
    return _CACHE["nc"]


def run_spmd(in_maps, **kwargs):
    from concourse.bass_utils import run_bass_kernel_spmd
    nc = get_program()
    return run_bass_kernel_spmd(nc, in_maps, list(range(NCORE)), **kwargs)


def kernel(x, Wq, Wk, Wv, Wp):
    in_maps = _host_prep(x, Wq, Wk, Wv, Wp)
    res = run_spmd(in_maps)
    out = np.zeros((B, S, O), np.float32)
    for c in range(NCORE):
        num = res.results[c]["out"]                    # [B, S, 128]
        den = res.results[c]["den"]                    # [B, S, 2]
        out += num[:, :, 0:64] / (den[:, :, 0:1] + EPS)
        out += num[:, :, 64:128] / (den[:, :, 1:2] + EPS)
    return out


# revision 15
# speedup vs baseline: 1.6008x; 1.0230x over previous
"""Trainium2 Bass kernel for MultiLinearAttention (causal linear attention).

Reference computation (per head h, feature map phi(u) = elu(u)+1):
    q = phi(x_h @ Wq_h), k = phi(x_h @ Wk_h), v = x_h @ Wv_h
    y_t = (q_t . sum_{s<=t} k_s v_s^T) / (q_t . sum_{s<=t} k_s + eps)
    out = concat_h(y_h) @ Wp

Sharding: 16 heads / 8 cores = 2 heads per core, all 4 batches per core.
Wp is folded per-head into the v projection (W'_h = Wv_h @ Wp_h); each core
ships per-head numerators [B, S, 2, 64] and denominators [B, S, 2]; the
host unshard computes y = sum_cores sum_h num_h / (den_h + eps).

Device algorithm: chunked causal linear attention, chunk C=128, all 4
batches fused per chunk into wide ops:
    u = 1 + [q|k] projections (PSUM preset via K=1 ones matmul)
    phi = max(u, min(exp(u-1), 1)) == elu(.)+1
    A^T = K Q^T per (b,h) (8 blocks, h-major: [4xh0 | 4xh1])
    am = A ⊙ causal-mask (one DVE op over all 8 blocks)
    num = am^T V + Q S_prev; den = am^T 1 + Q z_prev
    S += Kt^T V (PSUM-persistent); z via PSUM chunk-sums + SBUF f32 acc.

PSUM banks (8): state[512]f32 | u[1024]f32 x2 | A[1024]f32 x2 |
num[512]f32 | vk[512]f32 | {knp[512]bf16 + den/z [12]f32} shared.
"""

import os
import sys

import numpy as np

for _p in ("/root/.axon_site/_ro/trn_rl_repo", "/opt/trn_rl_repo", "/opt/pypackages"):
    if os.path.isdir(_p) and _p not in sys.path:
        sys.path.append(_p)

import ml_dtypes

B, S, D = 4, 4096, 1024
H, HD, O = 16, 64, 64
C = 128                  # chunk length
NCORE = 8
HPC = H // NCORE         # heads per core
NCHUNK = S // C
EPS = 1e-6

_CACHE = {}


def _build_program(nchunk=NCHUNK):
    import concourse.mybir as mybir
    from concourse import bacc
    from concourse.tile import TileContext

    fp32 = mybir.dt.float32
    cdt = mybir.dt.bfloat16
    Alu = mybir.AluOpType
    Act = mybir.ActivationFunctionType

    nc = bacc.Bacc()
    # x staged feature-major, chunk-interleaved: col = 512*chunk + 128*b + s
    xq_h = nc.declare_dram_parameter("xq", [128, 4 * S], cdt, isOutput=False)
    wq_h = nc.declare_dram_parameter("wq", [128, 128], cdt, isOutput=False)
    wk_h = nc.declare_dram_parameter("wk", [128, 128], cdt, isOutput=False)
    wv_h = nc.declare_dram_parameter("wv", [128, 128], cdt, isOutput=False)
    mask_h = nc.declare_dram_parameter("mask8", [128, 1024], cdt, isOutput=False)
    ident_h = nc.declare_dram_parameter("ident", [128, 128], cdt, isOutput=False)
    ones_h = nc.declare_dram_parameter("ones", [1, 512], cdt, isOutput=False)
    zer_h = nc.declare_dram_parameter("zer", [1, 512], cdt, isOutput=False)
    out_h = nc.declare_dram_parameter("out", [B, S, 128], fp32, isOutput=True)
    den_h = nc.declare_dram_parameter("den", [B, S, 2], fp32, isOutput=True)

    NXT = 8               # number of x SBUF tiles
    XCOLS = 4 * S // NXT  # 2048 cols per tile = 4 chunks

    with TileContext(nc) as tc:
        with (
            tc.tile_pool(name="consts", bufs=1) as consts,
            tc.tile_pool(name="work", bufs=2) as work,
            tc.tile_pool(name="stage", bufs=2) as stage,
            # PSUM pools; creation order fixes bank layout (8 banks total)
            tc.tile_pool(name="pu", bufs=1, space="PSUM") as pu,
            tc.tile_pool(name="pa", bufs=1, space="PSUM") as pa,
            tc.tile_pool(name="pn", bufs=1, space="PSUM") as pn,
            tc.tile_pool(name="psv", bufs=1, space="PSUM") as psv,
            tc.tile_pool(name="pkz", bufs=1, space="PSUM") as pkz,
        ):
            # ---- constants into SBUF ----
            wq = consts.tile([128, 128], cdt)
            wk = consts.tile([128, 128], cdt)
            wv = consts.tile([128, 128], cdt)
            mask8 = consts.tile([128, 1024], cdt)
            ident = consts.tile([128, 128], cdt)
            ones = consts.tile([1, 512], cdt)
            zer = consts.tile([1, 512], cdt)
            # SWDGE (gpsimd) for input loads keeps the SP sequencer free for
            # the batched output DMAs.
            nc.gpsimd.dma_start(wq, wq_h[:, :])
            nc.gpsimd.dma_start(wk, wk_h[:, :])
            nc.gpsimd.dma_start(wv, wv_h[:, :])
            nc.gpsimd.dma_start(mask8, mask_h[:, :])
            nc.gpsimd.dma_start(ident, ident_h[:, :])
            nc.gpsimd.dma_start(ones, ones_h[:, :])
            nc.gpsimd.dma_start(zer, zer_h[:, :])

            ones_col = consts.tile([128, 1], cdt)
            nc.gpsimd.memset(ones_col, 1.0)
            neg1 = consts.tile([128, 1], fp32)
            nc.gpsimd.memset(neg1, -1.0)

            xsb = []
            for t in range(NXT):
                xt = consts.tile([128, XCOLS], cdt, name=f"xsb{t}")
                nc.gpsimd.dma_start(xt, xq_h[:, t * XCOLS:(t + 1) * XCOLS])
                xsb.append(xt)

            def xchunk(i):
                """[128, 512] x columns of chunk i (4 batches)."""
                t, r = divmod(i * 512, XCOLS)
                return xsb[t][:, r:r + 512]

            # ping-pong SBUF state copies for den path
            s01z = [consts.tile([128, 8], cdt, name=f"s01z{j}") for j in range(2)]
            for t in s01z:
                nc.gpsimd.memset(t, 0.0)
            zsum = [consts.tile([128, 4], fp32, name=f"zsum{j}") for j in range(2)]

            # ---- persistent [state | vk] PSUM tile (2 banks) ----
            # state in bank A (accumulates forever, zeroed once); vk in bank
            # B (rewritten per chunk, start=True clears only its own bank).
            # One wide Act copy evacuates both as [s01v | vsb] bf16.
            sv = psv.tile([128, 1024], fp32, name="sv")
            state = sv[:, 0:512]
            vkreg = sv[:, 512:1024]
            nc.tensor.matmul(state, ones[:, 0:128], zer[:, 0:512],
                             start=True, stop=False, skip_group_check=True)

            def emit_uproj(i):
                """preset + q/k projections for chunk i -> u tile."""
                xc = xchunk(i)
                u = pu.tile([128, 1024], fp32, name="u")
                nc.tensor.matmul(u[:, 0:512], ones[:, 0:128], ones[:, 0:512],
                                 start=True, stop=False, skip_group_check=True)
                nc.tensor.matmul(u[:, 512:1024], ones[:, 0:128], ones[:, 0:512],
                                 start=True, stop=False, skip_group_check=True)
                nc.tensor.matmul(u[:, 0:512], wq, xc, start=False, stop=True,
                                 skip_group_check=True)
                nc.tensor.matmul(u[:, 512:1024], wk, xc, start=False, stop=True,
                                 skip_group_check=True)
                return u

            def emit_v(i):
                xc = xchunk(i)
                for b in range(4):
                    nc.tensor.matmul(vkreg[:, 128 * b:128 * (b + 1)],
                                     xc[:, 128 * b:128 * (b + 1)], wv,
                                     start=(b == 0), stop=(b == 3),
                                     skip_group_check=True)

            def emit_phi(u):
                """phi(u) = max(u, min(exp(u-1), 1)); u holds proj+1."""
                e2 = work.tile([128, 1024], cdt, name="e2")
                nc.scalar.activation(e2, u, Act.Exp, bias=neg1)
                phi2 = work.tile([128, 1024], cdt, name="phi2")
                nc.vector.scalar_tensor_tensor(phi2, e2, 1.0, u, Alu.min, Alu.max)
                return phi2

            # ---- prologue: chunk 0 front ----
            u = emit_uproj(0)
            emit_v(0)
            phi2 = emit_phi(u)
            comb = work.tile([128, 1024], cdt, name="comb")
            nc.scalar.copy(comb[:, 512:1024], vkreg)
            s01v_prev = None
            vsb = comb[:, 512:1024]
            numwide = denwide = None
            BCH = 8               # chunks per output-DMA batch

            for i in range(nchunk):
                if i % BCH == 0:
                    numwide = stage.tile([128, 512 * BCH], fp32, name="numwide")
                    denwide = stage.tile([128, 8 * BCH], fp32, name="denwide")

                # ---- next-chunk projections first: feeds exp->phi chain ----
                if i + 1 < nchunk:
                    u = emit_uproj(i + 1)

                # ---- A^T = K Q^T per (b,h); h-major blocks [4xh0 | 4xh1] ----
                a_ps = pa.tile([128, 1024], fp32, name="a_ps")
                for h in range(2):
                    es = slice(64 * h, 64 * (h + 1))
                    for b in range(4):
                        nc.tensor.matmul(
                            a_ps[:, 512 * h + 128 * b:512 * h + 128 * (b + 1)],
                            phi2[es, 512 + 128 * b:512 + 128 * (b + 1)],
                            phi2[es, 128 * b:128 * (b + 1)],
                            start=True, stop=True)
                # masked A -> SBUF (one wide DVE op)
                am2 = work.tile([128, 1024], cdt, name="am2")
                nc.vector.tensor_tensor(am2, a_ps, mask8, Alu.mult)

                if i + 1 < nchunk:
                    emit_v(i + 1)

                # ---- transpose phi(k) per batch -> token-major (bf16 PSUM) --
                # knp shares its bank with den/z; transposes must precede the
                # den writers of this chunk (PE order does that).
                kdz = pkz.tile([128, 544], cdt, name="kdz")
                knp = kdz[:, 0:512]
                denz = kdz[:, 512:536].bitcast(fp32)   # [128, 12] f32
                for b in range(4):
                    nc.tensor.transpose(
                        knp[:, 128 * b:128 * (b + 1)],
                        phi2[:, 512 + 128 * b:512 + 128 * (b + 1)], ident)
                knat = work.tile([128, 512], cdt, name="knat")
                nc.vector.tensor_copy(knat.bitcast(fp32), knp.bitcast(fp32))

                num = pn.tile([128, 512], fp32, name="num")

                # ---- cross-chunk terms: Q S_prev, Q z_prev ----
                if i > 0:
                    for b in range(4):
                        nc.tensor.matmul(
                            num[:, 128 * b:128 * (b + 1)],
                            phi2[:, 128 * b:128 * (b + 1)],
                            s01v_prev[:, 128 * b:128 * (b + 1)],
                            start=(b == 0), stop=False, skip_group_check=True)
                    for b in range(4):
                        nc.tensor.matmul(
                            denz[:, 2 * b:2 * b + 2],
                            phi2[:, 128 * b:128 * (b + 1)],
                            s01z[(i - 1) % 2][:, 2 * b:2 * b + 2],
                            start=(b == 0), stop=False, skip_group_check=True)

                # exp/phi for next chunk (Act+DVE overlap with PE below)
                if i + 1 < nchunk:
                    phi2_n = emit_phi(u)
                else:
                    phi2_n = None

                # ---- intra-chunk: num += am^T V, den += am^T 1 ----
                for h in range(2):
                    for b in range(4):
                        amb = am2[:, 512 * h + 128 * b:512 * h + 128 * (b + 1)]
                        nc.tensor.matmul(
                            num[:, 128 * b + 64 * h:128 * b + 64 * (h + 1)],
                            amb, vsb[:, 128 * b + 64 * h:128 * b + 64 * (h + 1)],
                            start=(i == 0 and h == 0 and b == 0), stop=True,
                            skip_group_check=True)
                        nc.tensor.matmul(
                            denz[:, 2 * b + h:2 * b + h + 1],
                            amb, ones_col,
                            start=(i == 0 and h == 0 and b == 0), stop=False,
                            skip_group_check=True)

                # ---- state update: S += Kt^T V; z_chunk = Kt^T 1 ----
                for h in range(2):
                    for b in range(4):
                        kt = knat[:, 128 * b + 64 * h:128 * b + 64 * (h + 1)]
                        nc.tensor.matmul(
                            state[64 * h:64 * (h + 1),
                                  128 * b + 64 * h:128 * b + 64 * (h + 1)],
                            kt, vsb[:, 128 * b + 64 * h:128 * b + 64 * (h + 1)],
                            start=False, stop=False, skip_group_check=True)
                for b in range(4):
                    nc.tensor.matmul(
                        denz[:, 8 + b:9 + b],
                        knat[:, 128 * b:128 * (b + 1)], ones_col,
                        start=False, stop=(b == 3), skip_group_check=True)

                # ---- [state | vk] -> SBUF for next chunk's cross terms ----
                if i + 1 < nchunk:
                    comb_n = work.tile([128, 1024], cdt, name="comb")
                    nc.scalar.copy(comb_n, sv)
                    s01v_prev = comb_n[:, 0:512]
                    vsb_n = comb_n[:, 512:1024]
                    zc = denz[:, 8:12]
                    if i == 0:
                        nc.vector.tensor_copy(zsum[0], zc)
                    else:
                        nc.vector.tensor_tensor(zsum[i % 2], zc,
                                                zsum[(i - 1) % 2], Alu.add)
                    szt = s01z[i % 2]
                    szv = szt.rearrange("p (g c) -> p g c", c=2)
                    zs = zsum[i % 2]
                    nc.gpsimd.tensor_copy(szv[0:64, :, 0:1], zs[0:64, :])
                    nc.gpsimd.tensor_copy(szv[64:128, :, 1:2], zs[64:128, :])
                else:
                    vsb_n = None

                # ---- evacuate num/den into wide staging; DMA every BCH ----
                ci = i % BCH
                nc.scalar.copy(numwide[:, 512 * ci:512 * (ci + 1)], num)
                nc.vector.tensor_copy(denwide[:, 8 * ci:8 * (ci + 1)],
                                      denz[:, 0:8])
                if ci == BCH - 1:
                    blk = slice((i - ci) * C, (i + 1) * C)
                    nwv = numwide.rearrange("p (c bo) -> p c bo", bo=512)
                    dwv = denwide.rearrange("p (c d) -> p c d", d=8)
                    for b in range(4):
                        nc.sync.dma_start(
                            out_h[b, blk, :].rearrange("(c s) o -> s c o",
                                                       s=128),
                            nwv[:, :, 128 * b:128 * (b + 1)])
                        nc.sync.dma_start(
                            den_h[b, blk, :].rearrange("(c s) d -> s c d",
                                                       s=128),
                            dwv[:, :, 2 * b:2 * b + 2])

                phi2 = phi2_n
                vsb = vsb_n

    nc.finalize()
    return nc


def _host_prep(x, Wq, Wk, Wv, Wp):
    """Shard inputs per core; returns in_maps list."""
    x = np.asarray(x, dtype=np.float32)
    Wq = np.asarray(Wq, dtype=np.float32)
    Wk = np.asarray(Wk, dtype=np.float32)
    Wv = np.asarray(Wv, dtype=np.float32)
    Wp = np.asarray(Wp, dtype=np.float32)
    ndt = ml_dtypes.bfloat16

    mask = np.triu(np.ones((C, C), np.float32))
    mask8 = np.tile(mask, (1, 8)).astype(ndt)          # [128, 1024]
    ident = np.eye(128, dtype=np.float32).astype(ndt)
    ones = np.ones((1, 512), np.float32).astype(ndt)
    zer = np.zeros((1, 512), np.float32).astype(ndt)

    in_maps = []
    for c in range(NCORE):
        h0 = HPC * c
        xs = x[:, :, 64 * h0:64 * (h0 + HPC)]          # [B, S, 128]
        xT = xs.transpose(2, 0, 1)                     # [128, B, S]
        # col = 512*chunk + 128*b + s_in_chunk
        xq = np.ascontiguousarray(
            xT.reshape(128, B, NCHUNK, C).transpose(0, 2, 1, 3)
        ).reshape(128, B * S).astype(ndt)
        wq_bd = np.zeros((128, 128), np.float32)
        wk_bd = np.zeros((128, 128), np.float32)
        wv_bd = np.zeros((128, 128), np.float32)
        for j in range(HPC):
            h = h0 + j
            sl = slice(64 * j, 64 * (j + 1))
            wq_bd[sl, sl] = Wq[h]
            wk_bd[sl, sl] = Wk[h]
            wv_bd[sl, sl] = Wv[h] @ Wp[64 * h:64 * (h + 1), :]
        in_maps.append({
            "xq": xq,
            "wq": wq_bd.astype(ndt),
            "wk": wk_bd.astype(ndt),
            "wv": wv_bd.astype(ndt),
            "mask8": mask8,
            "ident": ident,
            "ones": ones,
            "zer": zer,
        })
    return in_maps


def get_program():
    if "nc" not in _CACHE:
        _CACHE["nc"] = _build_program()
    return _CACHE["nc"]


def run_spmd(in_maps, **kwargs):
    from concourse.bass_utils import run_bass_kernel_spmd
    nc = get_program()
    return run_bass_kernel_spmd(nc, in_maps, list(range(NCORE)), **kwargs)


def kernel(x, Wq, Wk, Wv, Wp):
    in_maps = _host_prep(x, Wq, Wk, Wv, Wp)
    res = run_spmd(in_maps)
    out = np.zeros((B, S, O), np.float32)
    for c in range(NCORE):
        num = res.results[c]["out"]                    # [B, S, 128]
        den = res.results[c]["den"]                    # [B, S, 2]
        out += num[:, :, 0:64] / (den[:, :, 0:1] + EPS)
        out += num[:, :, 64:128] / (den[:, :, 1:2] + EPS)
    return out


# revision 19
# speedup vs baseline: 1.8418x; 1.1506x over previous
"""Trainium2 Bass kernel for MultiLinearAttention (causal linear attention).

Reference computation (per head h, feature map phi(u) = elu(u)+1):
    q = phi(x_h @ Wq_h), k = phi(x_h @ Wk_h), v = x_h @ Wv_h
    y_t = (q_t . sum_{s<=t} k_s v_s^T) / (q_t . sum_{s<=t} k_s + eps)
    out = concat_h(y_h) @ Wp

Sharding: 16 heads / 8 cores = 2 heads per core, all 4 batches per core.
Wp is folded per-head into the v projection (W'_h = Wv_h @ Wp_h); each core
ships per-head numerators [B, S, 2, 64] and denominators [B, S, 2]; the
host unshard computes y = sum_cores sum_h num_h / (den_h + eps).

Device algorithm: chunked causal linear attention, chunk C=128, all 4
batches fused per chunk into wide ops:
    u = 1 + [q|k] projections (PSUM preset via K=1 ones matmul)
    phi = max(u, min(exp(u-1), 1)) == elu(.)+1
    A^T = K Q^T per (b,h) (8 blocks, h-major: [4xh0 | 4xh1])
    am = A ⊙ causal-mask (one DVE op over all 8 blocks)
    num = am^T V + Q S_prev; den = am^T 1 + Q z_prev
    S += Kt^T V (PSUM-persistent); z via PSUM chunk-sums + SBUF f32 acc.

PSUM banks (8): state[512]f32 | u[1024]f32 x2 | A[1024]f32 x2 |
num[512]f32 | vk[512]f32 | {knp[512]bf16 + den/z [12]f32} shared.
"""

import os
import sys

import numpy as np

for _p in ("/root/.axon_site/_ro/trn_rl_repo", "/opt/trn_rl_repo", "/opt/pypackages"):
    if os.path.isdir(_p) and _p not in sys.path:
        sys.path.append(_p)

import ml_dtypes

B, S, D = 4, 4096, 1024
H, HD, O = 16, 64, 64
C = 128                  # chunk length
NCORE = 8
HPC = H // NCORE         # heads per core
NCHUNK = S // C
EPS = 1e-6

_CACHE = {}


def _build_program(nchunk=NCHUNK):
    import concourse.mybir as mybir
    from concourse import bacc
    from concourse.tile import TileContext

    fp32 = mybir.dt.float32
    cdt = mybir.dt.bfloat16
    Alu = mybir.AluOpType
    Act = mybir.ActivationFunctionType

    nc = bacc.Bacc()
    # x staged feature-major, chunk-interleaved: col = 512*chunk + 128*b + s
    xq_h = nc.declare_dram_parameter("xq", [128, 4 * S], cdt, isOutput=False)
    wq_h = nc.declare_dram_parameter("wq", [128, 128], cdt, isOutput=False)
    wk_h = nc.declare_dram_parameter("wk", [128, 128], cdt, isOutput=False)
    wv_h = nc.declare_dram_parameter("wv", [128, 128], cdt, isOutput=False)
    mask_h = nc.declare_dram_parameter("mask8", [128, 1024], cdt, isOutput=False)
    ident_h = nc.declare_dram_parameter("ident", [128, 128], cdt, isOutput=False)
    ones_h = nc.declare_dram_parameter("ones", [1, 512], cdt, isOutput=False)
    zer_h = nc.declare_dram_parameter("zer", [1, 512], cdt, isOutput=False)
    out_h = nc.declare_dram_parameter("out", [B, S, 128], fp32, isOutput=True)
    den_h = nc.declare_dram_parameter("den", [B, S, 2], fp32, isOutput=True)

    NXT = 8               # number of x SBUF tiles
    XCOLS = 4 * S // NXT  # 2048 cols per tile = 4 chunks

    with TileContext(nc) as tc:
        with (
            tc.tile_pool(name="consts", bufs=1) as consts,
            tc.tile_pool(name="work", bufs=2) as work,
            tc.tile_pool(name="stage", bufs=2) as stage,
            # PSUM pools; creation order fixes bank layout (8 banks total)
            tc.tile_pool(name="pu", bufs=2, space="PSUM") as pu,
            tc.tile_pool(name="pa", bufs=1, space="PSUM") as pa,
            tc.tile_pool(name="pn", bufs=1, space="PSUM") as pn,
            tc.tile_pool(name="psv", bufs=1, space="PSUM") as psv,
            tc.tile_pool(name="pkz", bufs=1, space="PSUM") as pkz,
        ):
            # ---- constants into SBUF ----
            wq = consts.tile([128, 128], cdt)
            wk = consts.tile([128, 128], cdt)
            wv = consts.tile([128, 128], cdt)
            mask8 = consts.tile([128, 1024], cdt)
            ident = consts.tile([128, 128], cdt)
            ones = consts.tile([1, 512], cdt)
            zer = consts.tile([1, 512], cdt)
            # SWDGE (gpsimd) for input loads keeps the SP sequencer free for
            # the batched output DMAs.
            nc.gpsimd.dma_start(wq, wq_h[:, :])
            nc.gpsimd.dma_start(wk, wk_h[:, :])
            nc.gpsimd.dma_start(wv, wv_h[:, :])
            nc.gpsimd.dma_start(mask8, mask_h[:, :])
            nc.gpsimd.dma_start(ident, ident_h[:, :])
            nc.gpsimd.dma_start(ones, ones_h[:, :])
            nc.gpsimd.dma_start(zer, zer_h[:, :])

            ones_col = consts.tile([128, 1], cdt)
            nc.gpsimd.memset(ones_col, 1.0)
            neg1 = consts.tile([128, 1], fp32)
            nc.gpsimd.memset(neg1, -1.0)

            xsb = []
            for t in range(NXT):
                xt = consts.tile([128, XCOLS], cdt, name=f"xsb{t}")
                nc.gpsimd.dma_start(xt, xq_h[:, t * XCOLS:(t + 1) * XCOLS])
                xsb.append(xt)

            def xchunk(i):
                """[128, 512] x columns of chunk i (4 batches)."""
                t, r = divmod(i * 512, XCOLS)
                return xsb[t][:, r:r + 512]

            # ping-pong SBUF state copies for den path
            s01z = [consts.tile([128, 8], cdt, name=f"s01z{j}") for j in range(2)]
            for t in s01z:
                nc.gpsimd.memset(t, 0.0)
            zsum = [consts.tile([128, 4], fp32, name=f"zsum{j}") for j in range(2)]

            # ---- persistent [state | vk] PSUM tile (2 banks) ----
            # state in bank A (accumulates forever, zeroed once); vk in bank
            # B (rewritten per chunk, start=True clears only its own bank).
            # One wide Act copy evacuates both as [s01v | vsb] bf16.
            sv = psv.tile([128, 1024], fp32, name="sv")
            state = sv[:, 0:512]
            vkreg = sv[:, 512:1024]
            nc.tensor.matmul(state, ones[:, 0:128], zer[:, 0:512],
                             start=True, stop=False, skip_group_check=True)

            def emit_uhalf(i, w):
                """preset + one projection (q or k) for chunk i -> [128,512].
                Separate q/k tiles double-buffer the u banks, breaking the
                phi(i) -> proj(i+1) -> exp(i+1) -> phi(i+1) serial ring."""
                xc = xchunk(i)
                u = pu.tile([128, 512], fp32, name="u", tag="u")
                nc.tensor.matmul(u, ones[:, 0:128], ones[:, 0:512],
                                 start=True, stop=False, skip_group_check=True)
                nc.tensor.matmul(u, w, xc, start=False, stop=True,
                                 skip_group_check=True)
                return u

            def emit_v(i):
                xc = xchunk(i)
                for b in range(4):
                    nc.tensor.matmul(vkreg[:, 128 * b:128 * (b + 1)],
                                     xc[:, 128 * b:128 * (b + 1)], wv,
                                     start=(b == 0), stop=(b == 3),
                                     skip_group_check=True)

            def emit_phi_half(u, nm):
                """phi(u) = max(u, min(exp(u-1), 1)); u holds proj+1."""
                e2 = work.tile([128, 512], cdt, name=f"e2{nm}", tag=f"e2{nm}")
                nc.scalar.activation(e2, u, Act.Exp, bias=neg1)
                ph = work.tile([128, 512], cdt, name=f"ph{nm}", tag=f"ph{nm}")
                nc.vector.scalar_tensor_tensor(ph, e2, 1.0, u, Alu.min, Alu.max)
                return ph

            # ---- prologue: chunk 0 front ----
            uq = emit_uhalf(0, wq)
            uk = emit_uhalf(0, wk)
            emit_v(0)
            phiq = emit_phi_half(uq, "q")
            phik = emit_phi_half(uk, "k")
            comb = work.tile([128, 1024], cdt, name="comb")
            nc.scalar.copy(comb[:, 512:1024], vkreg)
            s01v_prev = None
            vsb = comb[:, 512:1024]
            numwide = denwide = None
            BCH = 8               # chunks per output-DMA batch

            for i in range(nchunk):
                if i % BCH == 0:
                    numwide = stage.tile([128, 512 * BCH], fp32, name="numwide")
                    denwide = stage.tile([128, 8 * BCH], fp32, name="denwide")

                # ---- next-chunk projections first: feeds exp->phi chain ----
                if i + 1 < nchunk:
                    uq = emit_uhalf(i + 1, wq)
                    uk = emit_uhalf(i + 1, wk)

                # ---- A^T = K Q^T per (b,h); h-major blocks [4xh0 | 4xh1] ----
                a_ps = pa.tile([128, 1024], fp32, name="a_ps")
                for h in range(2):
                    es = slice(64 * h, 64 * (h + 1))
                    for b in range(4):
                        nc.tensor.matmul(
                            a_ps[:, 512 * h + 128 * b:512 * h + 128 * (b + 1)],
                            phik[es, 128 * b:128 * (b + 1)],
                            phiq[es, 128 * b:128 * (b + 1)],
                            start=True, stop=True)
                # masked A -> SBUF (one wide DVE op)
                am2 = work.tile([128, 1024], cdt, name="am2")
                nc.vector.tensor_tensor(am2, a_ps, mask8, Alu.mult)

                if i + 1 < nchunk:
                    emit_v(i + 1)

                # ---- transpose phi(k) per batch -> token-major (bf16 PSUM) --
                # knp shares its bank with den/z; transposes must precede the
                # den writers of this chunk (PE order does that).
                kdz = pkz.tile([128, 544], cdt, name="kdz")
                knp = kdz[:, 0:512]
                denz = kdz[:, 512:536].bitcast(fp32)   # [128, 12] f32
                for b in range(4):
                    nc.tensor.transpose(
                        knp[:, 128 * b:128 * (b + 1)],
                        phik[:, 128 * b:128 * (b + 1)], ident)
                knat = work.tile([128, 512], cdt, name="knat")
                nc.vector.tensor_copy(knat.bitcast(fp32), knp.bitcast(fp32))

                num = pn.tile([128, 512], fp32, name="num")

                # ---- cross-chunk terms: Q S_prev, Q z_prev ----
                if i > 0:
                    for b in range(4):
                        nc.tensor.matmul(
                            num[:, 128 * b:128 * (b + 1)],
                            phiq[:, 128 * b:128 * (b + 1)],
                            s01v_prev[:, 128 * b:128 * (b + 1)],
                            start=(b == 0), stop=False, skip_group_check=True)
                    for b in range(4):
                        nc.tensor.matmul(
                            denz[:, 2 * b:2 * b + 2],
                            phiq[:, 128 * b:128 * (b + 1)],
                            s01z[(i - 1) % 2][:, 2 * b:2 * b + 2],
                            start=(b == 0), stop=False, skip_group_check=True)

                # exp/phi for next chunk (Act+DVE overlap with PE below)
                if i + 1 < nchunk:
                    phiq_n = emit_phi_half(uq, "q")
                    phik_n = emit_phi_half(uk, "k")
                else:
                    phiq_n = phik_n = None

                # ---- intra-chunk: num += am^T V, den += am^T 1 ----
                for h in range(2):
                    for b in range(4):
                        amb = am2[:, 512 * h + 128 * b:512 * h + 128 * (b + 1)]
                        nc.tensor.matmul(
                            num[:, 128 * b + 64 * h:128 * b + 64 * (h + 1)],
                            amb, vsb[:, 128 * b + 64 * h:128 * b + 64 * (h + 1)],
                            start=(i == 0 and h == 0 and b == 0), stop=True,
                            skip_group_check=True)
                        nc.tensor.matmul(
                            denz[:, 2 * b + h:2 * b + h + 1],
                            amb, ones_col,
                            start=(i == 0 and h == 0 and b == 0), stop=False,
                            skip_group_check=True)

                # ---- state update: S += Kt^T V; z_chunk = Kt^T 1 ----
                for h in range(2):
                    for b in range(4):
                        kt = knat[:, 128 * b + 64 * h:128 * b + 64 * (h + 1)]
                        nc.tensor.matmul(
                            state[64 * h:64 * (h + 1),
                                  128 * b + 64 * h:128 * b + 64 * (h + 1)],
                            kt, vsb[:, 128 * b + 64 * h:128 * b + 64 * (h + 1)],
                            start=False, stop=False, skip_group_check=True)
                for b in range(4):
                    nc.tensor.matmul(
                        denz[:, 8 + b:9 + b],
                        knat[:, 128 * b:128 * (b + 1)], ones_col,
                        start=False, stop=(b == 3), skip_group_check=True)

                # ---- [state | vk] -> SBUF for next chunk's cross terms ----
                if i + 1 < nchunk:
                    comb_n = work.tile([128, 1024], cdt, name="comb")
                    nc.scalar.copy(comb_n, sv)
                    s01v_prev = comb_n[:, 0:512]
                    vsb_n = comb_n[:, 512:1024]
                    zc = denz[:, 8:12]
                    if i == 0:
                        nc.vector.tensor_copy(zsum[0], zc)
                    else:
                        nc.vector.tensor_tensor(zsum[i % 2], zc,
                                                zsum[(i - 1) % 2], Alu.add)
                    szt = s01z[i % 2]
                    szv = szt.rearrange("p (g c) -> p g c", c=2)
                    zs = zsum[i % 2]
                    nc.gpsimd.tensor_copy(szv[0:64, :, 0:1], zs[0:64, :])
                    nc.gpsimd.tensor_copy(szv[64:128, :, 1:2], zs[64:128, :])
                else:
                    vsb_n = None

                # ---- evacuate num/den into wide staging; DMA every BCH ----
                ci = i % BCH
                nc.scalar.copy(numwide[:, 512 * ci:512 * (ci + 1)], num)
                nc.vector.tensor_copy(denwide[:, 8 * ci:8 * (ci + 1)],
                                      denz[:, 0:8])
                if ci == BCH - 1:
                    blk = slice((i - ci) * C, (i + 1) * C)
                    nwv = numwide.rearrange("p (c bo) -> p c bo", bo=512)
                    dwv = denwide.rearrange("p (c d) -> p c d", d=8)
                    for b in range(4):
                        nc.sync.dma_start(
                            out_h[b, blk, :].rearrange("(c s) o -> s c o",
                                                       s=128),
                            nwv[:, :, 128 * b:128 * (b + 1)])
                        nc.sync.dma_start(
                            den_h[b, blk, :].rearrange("(c s) d -> s c d",
                                                       s=128),
                            dwv[:, :, 2 * b:2 * b + 2])

                phiq = phiq_n
                phik = phik_n
                vsb = vsb_n

    nc.finalize()
    return nc


def _host_prep(x, Wq, Wk, Wv, Wp):
    """Shard inputs per core; returns in_maps list."""
    x = np.asarray(x, dtype=np.float32)
    Wq = np.asarray(Wq, dtype=np.float32)
    Wk = np.asarray(Wk, dtype=np.float32)
    Wv = np.asarray(Wv, dtype=np.float32)
    Wp = np.asarray(Wp, dtype=np.float32)
    ndt = ml_dtypes.bfloat16

    mask = np.triu(np.ones((C, C), np.float32))
    mask8 = np.tile(mask, (1, 8)).astype(ndt)          # [128, 1024]
    ident = np.eye(128, dtype=np.float32).astype(ndt)
    ones = np.ones((1, 512), np.float32).astype(ndt)
    zer = np.zeros((1, 512), np.float32).astype(ndt)

    in_maps = []
    for c in range(NCORE):
        h0 = HPC * c
        xs = x[:, :, 64 * h0:64 * (h0 + HPC)]          # [B, S, 128]
        xT = xs.transpose(2, 0, 1)                     # [128, B, S]
        # col = 512*chunk + 128*b + s_in_chunk
        xq = np.ascontiguousarray(
            xT.reshape(128, B, NCHUNK, C).transpose(0, 2, 1, 3)
        ).reshape(128, B * S).astype(ndt)
        wq_bd = np.zeros((128, 128), np.float32)
        wk_bd = np.zeros((128, 128), np.float32)
        wv_bd = np.zeros((128, 128), np.float32)
        for j in range(HPC):
            h = h0 + j
            sl = slice(64 * j, 64 * (j + 1))
            wq_bd[sl, sl] = Wq[h]
            wk_bd[sl, sl] = Wk[h]
            wv_bd[sl, sl] = Wv[h] @ Wp[64 * h:64 * (h + 1), :]
        in_maps.append({
            "xq": xq,
            "wq": wq_bd.astype(ndt),
            "wk": wk_bd.astype(ndt),
            "wv": wv_bd.astype(ndt),
            "mask8": mask8,
            "ident": ident,
            "ones": ones,
            "zer": zer,
        })
    return in_maps


def get_program():
    if "nc" not in _CACHE:
        _CACHE["nc"] = _build_program()
    return _CACHE["nc"]


def run_spmd(in_maps, **kwargs):
    from concourse.bass_utils import run_bass_kernel_spmd
    nc = get_program()
    return run_bass_kernel_spmd(nc, in_maps, list(range(NCORE)), **kwargs)


def kernel(x, Wq, Wk, Wv, Wp):
    in_maps = _host_prep(x, Wq, Wk, Wv, Wp)
    res = run_spmd(in_maps)
    out = np.zeros((B, S, O), np.float32)
    for c in range(NCORE):
        num = res.results[c]["out"]                    # [B, S, 128]
        den = res.results[c]["den"]                    # [B, S, 2]
        out += num[:, :, 0:64] / (den[:, :, 0:1] + EPS)
        out += num[:, :, 64:128] / (den[:, :, 1:2] + EPS)
    return out
